# revision 18
# baseline (speedup 1.0000x reference)
"""Trainium2 Bass kernel for nn_Backbone GNN message-passing layer (v2).

Strategy (8 NeuronCores, SPMD, no collectives):
  - Balanced dst-node window packing: nodes are bin-packed into 392 windows
    of 128 slots (8 cores x 49 windows) equalizing per-window edge counts
    (split by src-half), so the uniform per-window edge capacity is ~6% over
    the mean instead of ~25%.  Core c owns windows [c*49, (c+1)*49).
  - Per-core node phase computes kvn = rsqrt(var)*(x @ [Wk|Wv]) + b for all
    nodes (bf16 table in DRAM) and qn for its own windows' slots.
    LayerNorm mean-centering is folded into the weights; variance comes from
    matmul columns (mean and E[x^2] via ones/H columns) on transposed-DMA
    loads, avoiding on-chip transposes and bn_stats.
  - Edge phase per window: K|V rows fetched with one dma_gather per
    (window, src-half) chunked at <=1024 indices; edge_attr projections add
    in PSUM with the per-edge LN scale fused via scalar_tensor_tensor.
    Q is never gathered: q_e = S @ q_win via the one-hot matmul (S built
    per-group with a broadcast tensor-tensor is_equal; S^T via PE transpose).
    Segment softmax/sum accumulate in PSUM over [128 nodes, 136] via
    matmuls against S.  exp max-subtraction is skipped (logits O(1)).
  - FFN (+residuals) per window out of PSUM as in v1.

Host-side preprocessing is limited to index/layout work: the window
bin-packing, edge bucketing/padding, permuting/transposing edge_attr,
int16 gather indices, and folding LN affine constants into weights.
"""

import os
import heapq
import numpy as np
import ml_dtypes
from contextlib import ExitStack

import concourse.bacc as bacc
import concourse.bass as bass
import concourse.tile as tile
import concourse.mybir as mybir
from concourse.bass_utils import run_bass_kernel_spmd

bf16 = ml_dtypes.bfloat16
F32 = mybir.dt.float32
BF = mybir.dt.bfloat16
I16 = mybir.dt.int16

N, E, H, NH, HD = 50000, 800000, 128, 8, 16
NCORES = 8
P = 128
NW = 49                       # windows per core
NWG = NCORES * NW             # 392 global windows
SPLIT = 32768
NODE_PAD = 50176
QROWS = NW * P                # 6272
EPS = 1e-5
GMAX = 1024                   # max indices per dma_gather call

AF = mybir.ActivationFunctionType
ALU = mybir.AluOpType


def _ceil(a, b):
    return -(-a // b)


def _wrap16(a):
    """[..., L] int16 -> [..., 128, L//16] gather-index layout."""
    sh = a.shape[:-1]
    L = a.shape[-1]
    w = a.reshape(*sh, L // 16, 16)
    w = np.swapaxes(w, -1, -2)
    reps = (1,) * len(sh) + (8, 1)
    return np.ascontiguousarray(np.tile(w, reps))


def _pack_windows(dega, degb):
    """Assign nodes to 392 windows of <=128 slots, balancing per-window
    A-edge and B-edge counts.  Returns perm[node] -> global slot."""
    tot = dega + degb
    order = np.argsort(-tot, kind="stable")
    ta = max(1.0, dega.sum() / NWG)
    tb = max(1.0, degb.sum() / NWG)
    wa = np.zeros(NWG)
    wb = np.zeros(NWG)
    nslots = np.zeros(NWG, np.int64)
    heap = [(0.0, w) for w in range(NWG)]
    heapq.heapify(heap)
    perm = np.empty(N, np.int64)
    for node in order:
        da, db = dega[node], degb[node]
        while True:
            key, w = heapq.heappop(heap)
            if nslots[w] < P:
                break
        perm[node] = w * P + nslots[w]
        nslots[w] += 1
        wa[w] += da
        wb[w] += db
        if nslots[w] < P:
            heapq.heappush(heap, (max(wa[w] / ta, wb[w] / tb), w))
    return perm, wa, wb


def _prep(inputs):
    x = np.asarray(inputs["x"], np.float32)
    ei = np.asarray(inputs["edge_index"])
    ea = np.asarray(inputs["edge_attr"], np.float32)
    f32 = np.float32
    Wq, Wk, Wv = (np.asarray(inputs[k], f32) for k in ("Wq", "Wk", "Wv"))
    Wek, Wev = (np.asarray(inputs[k], f32) for k in ("Wek", "Wev"))
    W1, W2 = np.asarray(inputs["W1"], f32), np.asarray(inputs["W2"], f32)
    bq, bk, bv = (np.asarray(inputs[k], f32) for k in ("bq", "bk", "bv"))
    bek, bev = (np.asarray(inputs[k], f32) for k in ("bek", "bev"))
    b1, b2 = np.asarray(inputs["b1"], f32), np.asarray(inputs["b2"], f32)
    lsw, lsb = np.asarray(inputs["ln_src_w"], f32), np.asarray(inputs["ln_src_b"], f32)
    lew, leb = np.asarray(inputs["ln_edge_w"], f32), np.asarray(inputs["ln_edge_b"], f32)
    lfw, lfb = np.asarray(inputs["ln_ffn_w"], f32), np.asarray(inputs["ln_ffn_b"], f32)

    src = ei[0].astype(np.int64)
    dst = ei[1].astype(np.int64)
    half = (src >= SPLIT).astype(np.int64)

    dega = np.bincount(dst[half == 0], minlength=N)
    degb = np.bincount(dst[half == 1], minlength=N)
    perm, wa, wb = _pack_windows(dega, degb)
    inv = np.full(NWG * P, -1, np.int64)
    inv[perm] = np.arange(N, dtype=np.int64)

    A_sub = max(1, _ceil(int(wa.max()), P))
    B_sub = max(1, _ceil(int(wb.max()), P))
    W_SUB = A_sub + B_sub
    AE, BE, WE = A_sub * P, B_sub * P, W_SUB * P
    E_pad = NW * WE
    S_total = NW * W_SUB

    sperm = perm[dst]
    wing = sperm >> 7
    rank = sperm & 127
    core = wing // NW
    wl = wing % NW
    group = (wing * 2 + half)
    NG = NWG * 2
    counts = np.bincount(group, minlength=NG)

    order = np.argsort(group, kind="stable")
    gs = group[order]
    starts = np.zeros(NG + 1, np.int64)
    np.cumsum(counts, out=starts[1:])
    within = np.arange(E, dtype=np.int64) - starts[gs]
    tgt = (gs // (2 * NW)) * E_pad + ((gs // 2) % NW) * WE + (gs & 1) * AE + within

    eid = np.full(NCORES * E_pad, -1, np.int64)
    eid[tgt] = order
    valid = eid >= 0
    eiv = eid[valid]

    ea_pad = np.zeros((NCORES * E_pad, H), bf16)
    ea_pad[valid] = ea.astype(bf16)[eiv]
    eaT = np.ascontiguousarray(
        ea_pad.reshape(NCORES, E_pad, H).transpose(0, 2, 1)
    )  # [8, 128, E_pad]

    kvidx = np.zeros(NCORES * E_pad, np.int64)
    kvidx[valid] = src[eiv] - SPLIT * half[eiv]
    kvidx = kvidx.astype(np.int16).reshape(NCORES, NW, WE)
    kvA = _wrap16(kvidx[:, :, :AE])
    kvB = _wrap16(kvidx[:, :, AE:])
    kvA = np.ascontiguousarray(kvA.transpose(0, 2, 1, 3))  # [8, 128, NW, AE//16]
    kvB = np.ascontiguousarray(kvB.transpose(0, 2, 1, 3))

    rk = np.full(NCORES * E_pad, 300.0, np.float32)
    rk[valid] = rank[eiv]
    rankpt = np.ascontiguousarray(
        rk.reshape(NCORES, S_total, P).transpose(0, 2, 1)
    )  # [8, 128, S_total]
    ar = np.arange(P, dtype=np.float32)
    S4h = (rankpt[:, :, :, None] == ar).astype(bf16).reshape(
        NCORES, P, S_total * P)                      # [8, 128, S_total*128]
    STwh = (rk.reshape(NCORES, 1, E_pad) == ar[None, :, None]).astype(bf16)
    STwh = np.ascontiguousarray(STwh)                # [8, 128, E_pad]

    x_bf = np.zeros((NODE_PAD, H), bf16)
    x_bf[:N] = x.astype(bf16)
    slot_node = inv.reshape(NCORES, QROWS)
    x_own_bf = np.zeros((NCORES, QROWS, H), bf16)
    x_own_f = np.zeros((NCORES, QROWS, H), np.float32)
    for c in range(NCORES):
        sn = slot_node[c]
        m = sn >= 0
        x_own_bf[c, m] = x_bf[sn[m]]
        x_own_f[c, m] = x[sn[m]]

    Cn = np.eye(H, dtype=f32) - np.full((H, H), 1.0 / H, f32)
    mcol = np.full((H, 1), 1.0 / H, f32)
    Wc_kv = np.concatenate([Cn @ (lsw[:, None] * Wk), Cn @ (lsw[:, None] * Wv)], 1)
    rhs_kv = np.concatenate([Wc_kv, mcol], 1).astype(bf16)          # [128, 257]
    Wc_ekv = np.concatenate([Cn @ (lew[:, None] * Wek), Cn @ (lew[:, None] * Wev)], 1)
    rhs_ekv = np.concatenate([Wc_ekv, mcol], 1).astype(bf16)        # [128, 257]
    rhs_q = np.concatenate([Cn @ (lsw[:, None] * Wq), mcol], 1).astype(bf16)
    onesdiv = mcol.astype(bf16)                                      # [128, 1]
    b_k = lsb @ Wk + bk + leb @ Wek + bek
    b_v = lsb @ Wv + bv + leb @ Wev + bev
    b_kv_rep = np.tile(np.concatenate([b_k, b_v])[None, :], (P, 1)).astype(bf16)
    b_q_rep = np.tile((lsb @ Wq + bq)[None, :], (P, 1)).astype(bf16)
    W1c = (Cn @ (lfw[:, None] * W1)).astype(bf16)
    b1_row = (lfb @ W1 + b1)[None, :].astype(bf16)
    W2p = np.ascontiguousarray(
        W2.reshape(4, P, H).transpose(1, 0, 2)
    ).astype(bf16)
    b2_row = b2[None, :].astype(bf16)

    ident = np.eye(P, dtype=f32).astype(bf16)
    ones_row = np.ones((1, P), bf16)

    shared = dict(
        x_bf=x_bf, rhs_kv=rhs_kv, rhs_ekv=rhs_ekv, rhs_q=rhs_q,
        onesdiv=onesdiv, b_kv_rep=b_kv_rep, b_q_rep=b_q_rep, W1c=W1c,
        b1_row=b1_row, W2p=W2p, b2_row=b2_row, ident=ident,
        ones_row=ones_row,
    )
    in_maps = []
    for c in range(NCORES):
        m = dict(shared)
        m.update(
            eaT=eaT[c], kvA=kvA[c], kvB=kvB[c],
            S4h=S4h[c], STwh=STwh[c],
            x_own_bf=x_own_bf[c], x_own_f=x_own_f[c],
        )
        in_maps.append(m)

    cfg = dict(A_sub=A_sub, B_sub=B_sub)
    return cfg, in_maps, perm


def _build(cfg):
    A_sub, B_sub = cfg["A_sub"], cfg["B_sub"]
    W_SUB = A_sub + B_sub
    AE, BE, WE = A_sub * P, B_sub * P, W_SUB * P
    E_pad = NW * WE
    S_total = NW * W_SUB

    nc = bacc.Bacc("TRN2", target_bir_lowering=False, debug=False)

    x_bf_d = nc.dram_tensor("x_bf", [NODE_PAD, H], BF, kind="ExternalInput")
    x_own_bf_d = nc.dram_tensor("x_own_bf", [QROWS, H], BF, kind="ExternalInput")
    x_own_f_d = nc.dram_tensor("x_own_f", [QROWS, H], F32, kind="ExternalInput")
    eaT_d = nc.dram_tensor("eaT", [P, E_pad], BF, kind="ExternalInput")
    kvA_d = nc.dram_tensor("kvA", [P, NW, AE // 16], I16, kind="ExternalInput")
    kvB_d = nc.dram_tensor("kvB", [P, NW, BE // 16], I16, kind="ExternalInput")
    rhs_kv_d = nc.dram_tensor("rhs_kv", [P, 257], BF, kind="ExternalInput")
    rhs_ekv_d = nc.dram_tensor("rhs_ekv", [P, 257], BF, kind="ExternalInput")
    rhs_q_d = nc.dram_tensor("rhs_q", [P, 129], BF, kind="ExternalInput")
    onesdiv_d = nc.dram_tensor("onesdiv", [P, 1], BF, kind="ExternalInput")
    b_kv_d = nc.dram_tensor("b_kv_rep", [P, 256], BF, kind="ExternalInput")
    b_q_d = nc.dram_tensor("b_q_rep", [P, P], BF, kind="ExternalInput")
    W1c_d = nc.dram_tensor("W1c", [P, 4 * H], BF, kind="ExternalInput")
    b1_d = nc.dram_tensor("b1_row", [1, 4 * H], BF, kind="ExternalInput")
    W2p_d = nc.dram_tensor("W2p", [P, 4, H], BF, kind="ExternalInput")
    b2_d = nc.dram_tensor("b2_row", [1, H], BF, kind="ExternalInput")
    S4h_d = nc.dram_tensor("S4h", [P, S_total * P], BF, kind="ExternalInput")
    STwh_d = nc.dram_tensor("STwh", [P, E_pad], BF, kind="ExternalInput")
    ident_d = nc.dram_tensor("ident", [P, P], BF, kind="ExternalInput")
    ones_d = nc.dram_tensor("ones_row", [1, P], BF, kind="ExternalInput")
    out_d = nc.dram_tensor("out", [QROWS, H], F32, kind="ExternalOutput")
    dbg = bool(os.environ.get("GNN_DEBUG", ""))
    if dbg:
        dbg_ekq = nc.dram_tensor("dbg_ekq", [P, 4, 386], BF, kind="ExternalOutput")
        dbg_kvf = nc.dram_tensor("dbg_kvf", [P, 4, 256], BF, kind="ExternalOutput")
        dbg_ew = nc.dram_tensor("dbg_ew", [P, 4, 8], BF, kind="ExternalOutput")
        dbg_uv = nc.dram_tensor("dbg_uv", [P, 20, H], BF, kind="ExternalOutput")
        dbg_ewa = nc.dram_tensor("dbg_ewa", [P, 20, 8], BF, kind="ExternalOutput")
        dbg_aggn = nc.dram_tensor("dbg_aggn", [P, H], F32, kind="ExternalOutput")
        dbg_den = nc.dram_tensor("dbg_den", [P, 8], F32, kind="ExternalOutput")
        dbg_xd = nc.dram_tensor("dbg_xd", [P, H], F32, kind="ExternalOutput")
        dbg_rs = nc.dram_tensor("dbg_rs", [P, 4, 1], F32, kind="ExternalOutput")

    with tile.TileContext(nc) as tc, ExitStack() as ctx:
        const = ctx.enter_context(tc.tile_pool(name="const", bufs=1))

        kvn_t = nc.dram_tensor("kvn_s", [NODE_PAD, 256], BF, kind="ExternalOutput")
        qn_t = nc.dram_tensor("qn_s", [QROWS, H], BF, kind="ExternalOutput")

        rhskv = const.tile([P, 257], BF)
        rhsekv = const.tile([P, 257], BF)
        rhsq = const.tile([P, 129], BF)
        onesdiv = const.tile([P, 1], BF)
        bkv = const.tile([P, 256], BF)
        bqr = const.tile([P, P], BF)
        w1c = const.tile([P, 4 * H], BF)
        b1r = const.tile([1, 4 * H], BF)
        w2p = const.tile([P, 4, H], BF)
        b2r = const.tile([1, H], BF)

        idn = const.tile([P, P], BF)
        onesr = const.tile([1, P], BF)
        kvA_sb = const.tile([P, NW, AE // 16], I16)
        kvB_sb = const.tile([P, NW, BE // 16], I16)
        eps_c = const.tile([P, 1], F32)
        nc.vector.memset(eps_c[:], EPS)
        for t, d in ((rhskv, rhs_kv_d), (rhsekv, rhs_ekv_d), (rhsq, rhs_q_d),
                     (onesdiv, onesdiv_d), (bkv, b_kv_d), (bqr, b_q_d),
                     (w1c, W1c_d), (b1r, b1_d), (w2p, W2p_d), (b2r, b2_d),
                     (idn, ident_d), (onesr, ones_d),
                     (kvA_sb, kvA_d), (kvB_sb, kvB_d)):
            nc.sync.dma_start(out=t[:], in_=d[:])

        # ---------------- node phase ----------------
        def project_nodes(x_dram, nsub, rhs_sb, wid, brep, dst_dram, tag):
            with ExitStack() as c2:
                sb = c2.enter_context(tc.tile_pool(name=f"np_{tag}", bufs=3))
                ps = c2.enter_context(
                    tc.tile_pool(name=f"npp_{tag}", bufs=4, space="PSUM"))
                for g in range(0, nsub, 16):
                    gn = min(16, nsub - g)
                    xT = sb.tile([P, 16, P], BF, tag="xT")
                    nc.sync.dma_start(
                        out=xT[:, 0:gn, :].rearrange("p m c -> p (m c)"),
                        in_=x_dram[g * P:(g + gn) * P, :], transpose=True)
                    xsq = sb.tile([P, 16, P], BF, tag="xsq")
                    nc.vector.tensor_mul(out=xsq[:, 0:gn, :], in0=xT[:, 0:gn, :],
                                         in1=xT[:, 0:gn, :])
                    stcp = sb.tile([P, 16, 2], F32, tag="stcp")
                    pstage = sb.tile([P, 16, wid], BF, tag="pstage")
                    for j in range(gn):
                        pp = ps.tile([P, wid + 2], F32, tag="pp")
                        nc.tensor.matmul(out=pp[:, 0:wid + 1], lhsT=xT[:, j, :],
                                         rhs=rhs_sb[:], start=True, stop=True)
                        nc.tensor.matmul(out=pp[:, wid + 1:wid + 2],
                                         lhsT=xsq[:, j, :], rhs=onesdiv[:],
                                         start=True, stop=True)
                        nc.vector.tensor_copy(out=stcp[:, j, :],
                                              in_=pp[:, wid:wid + 2])
                        nc.scalar.activation(out=pstage[:, j, :],
                                             in_=pp[:, 0:wid], func=AF.Copy)
                    m2 = sb.tile([P, 16, 1], F32, tag="m2")
                    nc.vector.tensor_mul(out=m2[:, 0:gn, :],
                                         in0=stcp[:, 0:gn, 0:1],
                                         in1=stcp[:, 0:gn, 0:1])
                    var = sb.tile([P, 16, 1], F32, tag="var")
                    nc.vector.tensor_sub(out=var[:, 0:gn, :],
                                         in0=stcp[:, 0:gn, 1:2],
                                         in1=m2[:, 0:gn, :])
                    sd = sb.tile([P, 16, 1], F32, tag="sd")
                    nc.scalar.activation(out=sd[:, 0:gn, :], in_=var[:, 0:gn, :],
                                         func=AF.Sqrt, bias=eps_c[:])
                    rs = sb.tile([P, 16, 1], F32, tag="rs")
                    nc.vector.reciprocal(out=rs[:, 0:gn, :], in_=sd[:, 0:gn, :])
                    stage = sb.tile([P, 16, wid], BF, tag="stage")
                    for j in range(gn):
                        nc.vector.scalar_tensor_tensor(
                            out=stage[:, j, :], in0=pstage[:, j, :],
                            scalar=rs[:, j, :], in1=brep[:, :wid],
                            op0=ALU.mult, op1=ALU.add)
                    nc.sync.dma_start(
                        out=dst_dram[g * P:(g + gn) * P, :].rearrange(
                            "(t p) c -> p t c", p=P),
                        in_=stage[:, 0:gn, :])

        project_nodes(x_bf_d, NODE_PAD // P, rhskv, 256, bkv, kvn_t, "kv")
        project_nodes(x_own_bf_d, NW, rhsq, H, bqr, qn_t, "q")

        # ---------------- edge phase ----------------
        with ExitStack() as c2:
            sbw = c2.enter_context(tc.tile_pool(name="win", bufs=2))
            sbg = c2.enter_context(tc.tile_pool(name="gat", bufs=3))
            sbe = c2.enter_context(tc.tile_pool(name="edge", bufs=2))
            sbf = c2.enter_context(tc.tile_pool(name="ffn", bufs=2))
            ps_proj = c2.enter_context(
                tc.tile_pool(name="pproj", bufs=4, space="PSUM"))
            ps_agg = c2.enter_context(
                tc.tile_pool(name="pagg", bufs=2, space="PSUM"))
            ps_ffn = c2.enter_context(
                tc.tile_pool(name="pffn", bufs=2, space="PSUM"))

            for w in range(NW):
                kv_g = sbg.tile([P, W_SUB, 256], BF, tag="kvg")
                c0 = 0
                while c0 < AE:
                    c1 = min(c0 + GMAX, AE)
                    nc.gpsimd.dma_gather(
                        kv_g[:, c0 // P:c1 // P, :], kvn_t[0:SPLIT, :],
                        kvA_sb[:, w, c0 // 16:c1 // 16], c1 - c0, c1 - c0, 256)
                    c0 = c1
                c0 = 0
                while c0 < BE:
                    c1 = min(c0 + GMAX, BE)
                    nc.gpsimd.dma_gather(
                        kv_g[:, A_sub + c0 // P:A_sub + c1 // P, :],
                        kvn_t[SPLIT:NODE_PAD, :],
                        kvB_sb[:, w, c0 // 16:c1 // 16], c1 - c0, c1 - c0, 256)
                    c0 = c1
                qwin = sbw.tile([P, P], BF, tag="qwin")
                nc.sync.dma_start(out=qwin[:], in_=qn_t[w * P:(w + 1) * P, :])
                ea_slab = sbw.tile([P, WE], BF, tag="eas")
                nc.sync.dma_start(out=ea_slab[:],
                                  in_=eaT_d[:, w * WE:(w + 1) * WE])
                agg = ps_agg.tile([P, 136], F32, tag="agg")

                # window-wide one-hot matrices (host-precomputed)
                S4 = sbw.tile([P, W_SUB, P], BF, tag="S4")
                nc.sync.dma_start(
                    out=S4[:].rearrange("p m c -> p (m c)"),
                    in_=S4h_d[:, w * WE:(w + 1) * WE])
                STw = sbw.tile([P, W_SUB, P], BF, tag="STw")
                nc.sync.dma_start(
                    out=STw[:].rearrange("p m c -> p (m c)"),
                    in_=STwh_d[:, w * WE:(w + 1) * WE])
                easq = sbw.tile([P, WE], BF, tag="easq")
                nc.vector.tensor_mul(out=easq[:], in0=ea_slab[:], in1=ea_slab[:])

                # pass A: per-subtile projections
                stcp = sbw.tile([P, W_SUB, 2], F32, tag="stcp")
                ekq = sbw.tile([P, W_SUB, 386], BF, tag="ekq")
                for sub in range(W_SUB):
                    proj = ps_proj.tile([P, 386], F32, tag="proj")
                    nc.tensor.matmul(out=proj[:, 0:257],
                                     lhsT=ea_slab[:, sub * P:(sub + 1) * P],
                                     rhs=rhsekv[:], start=True, stop=True)
                    nc.tensor.matmul(out=proj[:, 257:258],
                                     lhsT=easq[:, sub * P:(sub + 1) * P],
                                     rhs=onesdiv[:], start=True, stop=True)
                    nc.tensor.matmul(out=proj[:, 258:386], lhsT=STw[:, sub, :],
                                     rhs=qwin[:], start=True, stop=True)
                    nc.scalar.activation(out=stcp[:, sub, :],
                                         in_=proj[:, 256:258], func=AF.Copy)
                    nc.scalar.activation(out=ekq[:, sub, :],
                                         in_=proj[:, 0:386], func=AF.Copy)

                # pass B: window-wide math
                m2 = sbe.tile([P, W_SUB, 1], F32, tag="m2")
                nc.vector.tensor_mul(out=m2[:], in0=stcp[:, :, 0:1],
                                     in1=stcp[:, :, 0:1])
                var = sbe.tile([P, W_SUB, 1], F32, tag="var")
                nc.vector.tensor_sub(out=var[:], in0=stcp[:, :, 1:2], in1=m2[:])
                sd = sbe.tile([P, W_SUB, 1], F32, tag="sd")
                nc.scalar.activation(out=sd[:], in_=var[:], func=AF.Sqrt,
                                     bias=eps_c[:])
                rs = sbe.tile([P, W_SUB, 1], F32, tag="rs")
                nc.vector.reciprocal(out=rs[:], in_=sd[:])
                kvf = sbe.tile([P, W_SUB, 256], BF, tag="kvf")
                for sub in range(W_SUB):
                    nc.vector.scalar_tensor_tensor(
                        out=kvf[:, sub, :], in0=ekq[:, sub, 0:256],
                        scalar=rs[:, sub, :], in1=kv_g[:, sub, :],
                        op0=ALU.mult, op1=ALU.add)
                qk = sbe.tile([P, W_SUB, P], BF, tag="qk")
                nc.vector.tensor_mul(out=qk[:], in0=ekq[:, :, 258:386],
                                     in1=kvf[:, :, 0:H])
                wl = sbe.tile([P, W_SUB, NH], F32, tag="wl")
                nc.vector.tensor_reduce(
                    out=wl[:],
                    in_=qk[:].rearrange("p m (h d) -> p m h d", d=HD),
                    axis=mybir.AxisListType.X, op=ALU.add)
                U = sbe.tile([P, W_SUB, 136], BF, tag="U")
                nc.scalar.activation(out=U[:, :, H:136], in_=wl[:],
                                     func=AF.Exp, scale=0.25)
                nc.vector.tensor_mul(
                    out=U[:, :, 0:H].rearrange("p m (h d) -> p m h d", d=HD),
                    in0=kvf[:, :, H:256].rearrange("p m (h d) -> p m h d", d=HD),
                    in1=U[:, :, H:136].unsqueeze(3).broadcast_to(
                        [P, W_SUB, NH, HD]))
                for sub in range(W_SUB):
                    nc.tensor.matmul(out=agg[:], lhsT=S4[:, sub, :],
                                     rhs=U[:, sub, :],
                                     start=(sub == 0), stop=(sub == W_SUB - 1))

                # ---- finalize + FFN ----
                den = sbf.tile([P, NH], F32, tag="den")
                nc.scalar.activation(out=den[:], in_=agg[:, H:136],
                                     func=AF.Copy, bias=1e-16)
                rden = sbf.tile([P, NH], F32, tag="rden")
                nc.vector.reciprocal(out=rden[:], in_=den[:])
                xw = sbf.tile([P, H], F32, tag="xw")
                nc.sync.dma_start(out=xw[:], in_=x_own_f_d[w * P:(w + 1) * P, :])
                aggn = sbf.tile([P, H], F32, tag="aggn")
                nc.vector.tensor_mul(
                    out=aggn[:].rearrange("p (h d) -> p h d", d=HD),
                    in0=agg[:, 0:H].rearrange("p (h d) -> p h d", d=HD),
                    in1=rden[:].unsqueeze(2).broadcast_to([P, NH, HD]))
                xd = sbf.tile([P, H], F32, tag="xd")
                nc.vector.tensor_add(out=xd[:], in0=xw[:], in1=aggn[:])
                if dbg and w == 0:
                    nc.sync.dma_start(out=dbg_den[:], in_=den[:])
                    nc.sync.dma_start(out=dbg_xd[:], in_=xd[:])
                    nc.sync.dma_start(out=dbg_aggn[:], in_=aggn[:])

                st6f = sbf.tile([P, 6], F32, tag="st6f")
                mvf = sbf.tile([P, 2], F32, tag="mvf")
                nc.vector.bn_stats(out=st6f[:], in_=xd[:])
                nc.vector.bn_aggr(out=mvf[:], in_=st6f[:])
                sdf = sbf.tile([P, 1], F32, tag="sdf")
                nc.scalar.activation(out=sdf[:], in_=mvf[:, 1:2], func=AF.Sqrt,
                                     bias=eps_c[:])
                rsf = sbf.tile([P, 1], F32, tag="rsf")
                nc.vector.reciprocal(out=rsf[:], in_=sdf[:])
                hp = sbf.tile([P, H], BF, tag="hp")
                nc.vector.tensor_scalar_mul(out=hp[:], in0=xd[:], scalar1=rsf[:])
                hT_ps = ps_ffn.tile([P, 4 * H], BF, tag="fps")
                nc.tensor.transpose(out=hT_ps[:, 0:P], in_=hp[:], identity=idn[:])
                hT = sbf.tile([P, P], BF, tag="hT")
                nc.scalar.activation(out=hT[:], in_=hT_ps[:, 0:P], func=AF.Copy)
                h1 = ps_ffn.tile([P, 4 * H], F32, tag="fps")
                nc.tensor.matmul(out=h1[:], lhsT=hT[:], rhs=w1c[:],
                                 start=True, stop=False)
                nc.tensor.matmul(out=h1[:], lhsT=onesr[:], rhs=b1r[:],
                                 start=False, stop=True)
                r = sbf.tile([P, 4 * H], BF, tag="r")
                nc.scalar.activation(out=r[:], in_=h1[:], func=AF.Relu)
                rT_ps = ps_ffn.tile([P, 4 * H], BF, tag="fps")
                for k in range(4):
                    nc.tensor.transpose(out=rT_ps[:, k * P:(k + 1) * P],
                                        in_=r[:, k * P:(k + 1) * P],
                                        identity=idn[:])
                rT = sbf.tile([P, 4 * H], BF, tag="rT")
                nc.scalar.activation(out=rT[:], in_=rT_ps[:], func=AF.Copy)
                op = ps_ffn.tile([P, 4 * H], F32, tag="fps")
                for k in range(4):
                    nc.tensor.matmul(out=op[:, 0:H], lhsT=rT[:, k * P:(k + 1) * P],
                                     rhs=w2p[:, k, :], start=(k == 0), stop=False)
                nc.tensor.matmul(out=op[:, 0:H], lhsT=onesr[:], rhs=b2r[:],
                                 start=False, stop=True)
                ob = sbf.tile([P, H], F32, tag="ob")
                nc.vector.tensor_add(out=ob[:], in0=xd[:], in1=op[:, 0:H])
                nc.sync.dma_start(out=out_d[w * P:(w + 1) * P, :], in_=ob[:])

    nc.compile()
    return nc


_CACHE = {}


def _get_program(cfg):
    key = tuple(sorted(cfg.items()))
    if key not in _CACHE:
        _CACHE[key] = _build(cfg)
    return _CACHE[key]


def kernel(_collect_results=None, **inputs):
    cfg, in_maps, perm = _prep(inputs)
    nc = _get_program(cfg)
    res = run_bass_kernel_spmd(
        nc, in_maps, core_ids=list(range(NCORES)),
        trace=bool(os.environ.get("GNN_TRACE", "")))
    if _collect_results is not None:
        _collect_results.append(res)
    full = np.concatenate([res.results[c]["out"] for c in range(NCORES)], 0)
    return np.ascontiguousarray(full[perm])
